# revision 16
# baseline (speedup 1.0000x reference)
"""Trainium2 Bass kernel for nn_FFTCNN (FFT-conv text classifier).

Math: the reference's fft_conv1d (irfft(rfft(x) * rfft(w_pad))) is exactly a
K=3 circular convolution.  conv1 is linear in the embedding, so the host
precomputes the fused table EW[t, k*128+o] = (emb @ w1[:,:,k].T)[t, o] and
the device gathers conv1's output contributions directly - no conv1 matmuls
and 25% fewer gathered bytes than raw embeddings.

Sharding: data-parallel over batch - 8 cores x 4 batch elements; tables and
weights replicated.

Per-core pipeline (per batch element):
  1. dma_gather(transpose=True) on EW (fp16, elem 384) gathers token rows
     AND transposes them into xt[o, k, pos] layout in one DMA.  Circular
     wrap handled by gathering positions [l0-2 .. l0+CSPAN) per chunk, so
     shifted reads never cross tiles.  Chunked (GCHUNK/batch) under the
     SWDGE 64-descriptor/engine packet ceiling, spread over 4 SWDGE queues.
  2. "conv1" = 3 identity matmuls accumulating the k-shifted slices in
     PSUM, then ReLU+bias on ScalarE into an extended h1 buffer (fp16).
  3. conv2: per l-tile 3 shift matmuls -> PSUM, reduce_max per tile
     (max-pool commutes with the monotonic bias+ReLU).
  4. pooled -> relu(. + b2) -> small MLP on-device -> [CLASSES, 4] out.
"""

import os
import sys

sys.path.insert(0, "/opt/trn_rl_repo")

import numpy as np

B, L = 32, 4096
VOCAB, EMB, HID, CLASSES = 20000, 512, 128, 6
K = 3
NCORES = 8
BLOC = B // NCORES          # batch elements per core
LTILE = 512
NLT = L // LTILE            # 8 l-tiles
LEXT = L + 2                # extended h1 columns
EWC = K * HID               # fused table row length (384)

GCHUNK = int(os.environ.get("KERNEL_GCHUNK", "8"))
SINGLE_PACKET = os.environ.get("KERNEL_SINGLE_PACKET", "1") == "1"
NQUEUES = int(os.environ.get("KERNEL_NQUEUES", "4"))
XTBUFS = int(os.environ.get("KERNEL_XTBUFS", str(3 * GCHUNK)))
CSPAN = L // GCHUNK
CNIDX = CSPAN + 128
NVALID = CSPAN + 2   # real indices per chunk; the rest are -1 (ucode trims)
# HW limit: the single-packet tx stream allows <=64 descriptors/engine,
# i.e. num_idxs/16 + 2 <= 64 -> num_idxs <= 992 per dma_gather.
assert CNIDX <= 992


def _dtype_np(mode):
    import ml_dtypes

    return np.float16 if mode == "f16" else ml_dtypes.bfloat16


def _dtype_my(mode):
    import concourse.mybir as mybir

    return mybir.dt.float16 if mode == "f16" else mybir.dt.bfloat16


def build_program(mode="f16", nbatch=BLOC):
    """Build the per-core Bass program."""
    import concourse.bacc as bacc
    import concourse.mybir as mybir
    import concourse.tile as tile
    from concourse._compat import get_trn_type

    f32 = mybir.dt.float32
    f16 = _dtype_my(mode)
    i16 = mybir.dt.int16
    RELU = mybir.ActivationFunctionType.Relu
    IDENT = mybir.ActivationFunctionType.Identity
    AX = mybir.AxisListType.X

    nc = bacc.Bacc(
        get_trn_type() or "TRN2",
        target_bir_lowering=False,
        debug=False,
        enable_asserts=False,
        num_devices=NCORES,
        num_swdge_queues=NQUEUES,
        # ring capacity per queue = scratch/16 descs; all 8 gathers of a
        # queue (134 s2m descs each) must fit to avoid reclaim stalls.
        dynamic_dma_scratch_size=49152,
    )

    ncols = CNIDX // 16  # idx columns per gather chunk

    ew_d = nc.dram_tensor("ew", [VOCAB, EWC], f16, kind="ExternalInput")
    idx_d = nc.dram_tensor("idx", [128, nbatch * GCHUNK * ncols], i16,
                           kind="ExternalInput")
    id_d = nc.dram_tensor("id128", [128, 128], f16, kind="ExternalInput")
    w2_d = nc.dram_tensor("w2t", [128, K * HID], f16, kind="ExternalInput")
    b1_d = nc.dram_tensor("b1c", [128, 1], f32, kind="ExternalInput")
    b2_d = nc.dram_tensor("b2c", [128, 1], f32, kind="ExternalInput")
    lw1_d = nc.dram_tensor("lw1t", [128, HID], f16, kind="ExternalInput")
    lb1_d = nc.dram_tensor("lb1c", [128, 1], f32, kind="ExternalInput")
    lw2_d = nc.dram_tensor("lw2t", [128, CLASSES], f16, kind="ExternalInput")
    lb2_d = nc.dram_tensor("lb2c", [CLASSES, 1], f32, kind="ExternalInput")
    out_d = nc.dram_tensor("out", [CLASSES, nbatch], f32, kind="ExternalOutput")

    from concourse import library_config

    with tile.TileContext(nc) as tc:
        # preload the GPSIMD ucode library that dma_gather needs, so its
        # ~9us IRAM fetch overlaps the preamble instead of stalling the
        # first gather.
        nc.gpsimd.load_library(library_config.mlp)
        with (
            tc.tile_pool(name="const", bufs=1) as cpool,
            tc.tile_pool(name="xt", bufs=XTBUFS) as xt_pool,
            tc.tile_pool(name="h1", bufs=2) as h1_pool,
            tc.tile_pool(name="small", bufs=2) as sm_pool,
            tc.tile_pool(name="ps", bufs=4, space="PSUM") as ps_pool,
            tc.tile_pool(name="psm", bufs=1, space="PSUM") as psm_pool,
        ):
            id_sb = cpool.tile([128, 128], f16)
            nc.sync.dma_start(id_sb[:, :], id_d.ap())
            w2_sb = cpool.tile([128, K * HID], f16)
            nc.sync.dma_start(w2_sb[:, :], w2_d.ap())
            lw1_sb = cpool.tile([128, HID], f16)
            nc.sync.dma_start(lw1_sb[:, :], lw1_d.ap())
            lw2_sb = cpool.tile([128, CLASSES], f16)
            nc.sync.dma_start(lw2_sb[:, :], lw2_d.ap())
            b1_sb = cpool.tile([128, 1], f32)
            nc.sync.dma_start(b1_sb[:, :], b1_d.ap())
            b2_sb = cpool.tile([128, 1], f32)
            nc.sync.dma_start(b2_sb[:, :], b2_d.ap())
            lb1_sb = cpool.tile([128, 1], f32)
            nc.sync.dma_start(lb1_sb[:, :], lb1_d.ap())
            lb2_sb = cpool.tile([CLASSES, 1], f32)
            nc.sync.dma_start(lb2_sb[:, :], lb2_d.ap())
            idx_sb = cpool.tile([128, nbatch * GCHUNK * ncols], i16)
            nc.sync.dma_start(idx_sb[:, :], idx_d.ap())

            y_sb = cpool.tile([128, nbatch], f16, tag="ytile")

            for b in range(nbatch):
                xts = []
                for c in range(GCHUNK):
                    xt = xt_pool.tile([128, K, CNIDX], f16, tag="xt")
                    o = (b * GCHUNK + c) * ncols
                    nc.gpsimd.dma_gather(
                        out_ap=xt[:, :, :],
                        in_ap=ew_d.ap(),
                        idxs_ap=idx_sb[:, o : o + ncols],
                        num_idxs=CNIDX,
                        num_idxs_reg=NVALID,
                        elem_size=EWC,
                        transpose=True,
                        single_packet=SINGLE_PACKET,
                        queue_num=(b * GCHUNK + c) % NQUEUES,
                    )
                    xts.append(xt)

                h1 = h1_pool.tile([128, LEXT], f16, tag="h1")
                for lt in range(NLT):
                    l0 = lt * LTILE
                    xt = xts[l0 // CSPAN]
                    base = l0 % CSPAN
                    ps1 = ps_pool.tile([128, LTILE], f32, tag="ps")
                    # conv1 output = sum of k-shifted gathered slices
                    for k in range(K):
                        nc.tensor.matmul(
                            ps1[:, :],
                            lhsT=id_sb[:, :],
                            rhs=xt[:, k, base + 2 - k : base + 2 - k + LTILE],
                            start=(k == 0),
                            stop=(k == K - 1),
                        )
                    nc.scalar.activation(
                        h1[:, 2 + l0 : 2 + l0 + LTILE], ps1[:, :], RELU,
                        bias=b1_sb[:, 0:1],
                    )

                # circular wrap: h1[-1], h1[-2] live at ext cols 1, 0
                nc.scalar.copy(h1[:, 0:2], h1[:, L : L + 2])

                mx = sm_pool.tile([128, NLT], f32, tag="mx")
                # tile 0 last: it needs the wrap columns, which depend on
                # conv1's final tile - every other tile is ready earlier
                for lt in list(range(1, NLT)) + [0]:
                    l0 = lt * LTILE
                    ps2 = ps_pool.tile([128, LTILE], f32, tag="ps")
                    for k in range(K):
                        nc.tensor.matmul(
                            ps2[:, :],
                            lhsT=w2_sb[:, k * HID : (k + 1) * HID],
                            rhs=h1[:, l0 + 2 - k : l0 + 2 - k + LTILE],
                            start=(k == 0),
                            stop=(k == K - 1),
                        )
                    nc.vector.reduce_max(mx[:, lt : lt + 1], ps2[:, :], axis=AX)

                pooled = sm_pool.tile([128, 1], f32, tag="pooled")
                nc.vector.reduce_max(pooled[:, :], mx[:, :], axis=AX)
                # max-pool commutes with (+b2, relu); cast to f16 for the MLP
                nc.scalar.activation(
                    y_sb[:, b : b + 1], pooled[:, :], RELU, bias=b2_sb[:, 0:1]
                )

            # --- tiny MLP head on all nbatch columns at once ---
            psm1 = psm_pool.tile([128, nbatch], f32, tag="psm1")
            nc.tensor.matmul(psm1[:, :], lhsT=lw1_sb[:, :], rhs=y_sb[:, :],
                             start=True, stop=True)
            z1 = sm_pool.tile([128, nbatch], f16, tag="z1")
            nc.scalar.activation(z1[:, :], psm1[:, :], RELU, bias=lb1_sb[:, 0:1])

            psm2 = psm_pool.tile([CLASSES, nbatch], f32, tag="psm2")
            nc.tensor.matmul(psm2[:, :], lhsT=lw2_sb[:, :], rhs=z1[:, :],
                             start=True, stop=True)
            out_sb = sm_pool.tile([CLASSES, nbatch], f32, tag="osb")
            nc.scalar.activation(out_sb[:, :], psm2[:, :], IDENT,
                                 bias=lb2_sb[:, 0:1])
            nc.sync.dma_start(out_d.ap(), out_sb[:, :])

    nc.compile()
    return nc


def prep_host_inputs(tokens, emb, w1, b1, w2, b2, lw1, lb1, lw2, lb2,
                     mode="f16", nbatch=BLOC):
    """Host-side layout prep.  Returns per-core in_maps."""
    npdt = _dtype_np(mode)
    tokens = np.asarray(tokens).astype(np.int64)
    emb = np.asarray(emb, np.float32)
    w1 = np.asarray(w1, np.float32)               # [HID, EMB, K]

    # fused conv1 table: ew[t, k*HID + o] = sum_c emb[t, c] * w1[o, c, k]
    ew = np.empty((VOCAB, EWC), np.float32)
    for k in range(K):
        ew[:, k * HID : (k + 1) * HID] = emb @ w1[:, :, k].T
    ew = np.ascontiguousarray(ew.astype(npdt))

    # w2t[p, k*HID + o] = w2[o, p, k]
    w2 = np.asarray(w2, np.float32)               # [HID, HID, K]
    w2t = np.ascontiguousarray(
        w2.transpose(1, 2, 0).reshape(128, K * HID).astype(npdt)
    )
    lw1t = np.ascontiguousarray(np.asarray(lw1, np.float32).T.astype(npdt))
    lw2t = np.ascontiguousarray(np.asarray(lw2, np.float32).T.astype(npdt))
    b1c = np.asarray(b1, np.float32).reshape(128, 1)
    b2c = np.asarray(b2, np.float32).reshape(128, 1)
    lb1c = np.asarray(lb1, np.float32).reshape(128, 1)
    lb2c = np.asarray(lb2, np.float32).reshape(CLASSES, 1)
    id128 = np.eye(128, dtype=np.float32).astype(npdt)

    pos = np.arange(CNIDX)
    in_maps = []
    for c in range(NCORES):
        idx_cols = []
        for j in range(nbatch):
            t = tokens[c * BLOC + j]
            for g in range(GCHUNK):
                ext = t[(g * CSPAN - 2 + pos) % L].astype(np.int16)
                ext[NVALID:] = -1  # ucode trims trailing -1s: no wasted bytes
                # gathered row i reads idxs[16, ncols] at [i % 16, i // 16]
                wrapped = ext.reshape(CNIDX // 16, 16).T
                idx_cols.append(np.tile(wrapped, (8, 1)))      # [128, ncols]
        idx = np.ascontiguousarray(np.concatenate(idx_cols, axis=1))
        in_maps.append({
            "ew": ew, "idx": idx, "id128": id128, "w2t": w2t,
            "b1c": b1c, "b2c": b2c, "lw1t": lw1t, "lb1c": lb1c,
            "lw2t": lw2t, "lb2c": lb2c,
        })
    return in_maps


_CACHE = {}


def _get_program(mode):
    if mode not in _CACHE:
        _CACHE[mode] = build_program(mode)
    return _CACHE[mode]


def run(inputs, mode=None, trace=False, trace_kwargs=None):
    """Run on 8 cores; returns (output[32, 6] f32, BassKernelResults)."""
    from concourse import bass_utils

    mode = mode or os.environ.get("KERNEL_MODE", "f16")
    nc = _get_program(mode)
    in_maps = prep_host_inputs(**inputs, mode=mode)
    res = bass_utils.run_bass_kernel_spmd(
        nc, in_maps, core_ids=list(range(NCORES)), trace=trace,
        **(trace_kwargs or {}),
    )
    out = np.empty((B, CLASSES), np.float32)
    for c in range(NCORES):
        o = res.results[c]["out"]  # [CLASSES, BLOC]
        out[c * BLOC : (c + 1) * BLOC, :] = np.asarray(o, np.float32).T
    return out, res


def kernel(**inputs):
    out, _ = run(inputs)
    return out


# revision 17
# speedup vs baseline: 1.0141x; 1.0141x over previous
"""Trainium2 Bass kernel for nn_FFTCNN (FFT-conv text classifier).

Math: the reference's fft_conv1d (irfft(rfft(x) * rfft(w_pad))) is exactly a
K=3 circular convolution.  conv1 is linear in the embedding, so the host
precomputes the fused table EW[t, k*128+o] = (emb @ w1[:,:,k].T)[t, o] and
the device gathers conv1's output contributions directly - no conv1 matmuls
and 25% fewer gathered bytes than raw embeddings.

Sharding: data-parallel over batch - 8 cores x 4 batch elements; tables and
weights replicated.

Per-core pipeline (per batch element):
  1. dma_gather(transpose=True) on EW (fp16, elem 384) gathers token rows
     AND transposes them into xt[o, k, pos] layout in one DMA.  Circular
     wrap handled by gathering positions [l0-2 .. l0+CSPAN) per chunk, so
     shifted reads never cross tiles.  Chunked (GCHUNK/batch) under the
     SWDGE 64-descriptor/engine packet ceiling, spread over 4 SWDGE queues.
  2. "conv1" = 3 identity matmuls accumulating the k-shifted slices in
     PSUM, then ReLU+bias on ScalarE into an extended h1 buffer (fp16).
  3. conv2: per l-tile 3 shift matmuls -> PSUM, reduce_max per tile
     (max-pool commutes with the monotonic bias+ReLU).
  4. pooled -> relu(. + b2) -> small MLP on-device -> [CLASSES, 4] out.
"""

import os
import sys

sys.path.insert(0, "/opt/trn_rl_repo")

import numpy as np

B, L = 32, 4096
VOCAB, EMB, HID, CLASSES = 20000, 512, 128, 6
K = 3
NCORES = 8
BLOC = B // NCORES          # batch elements per core
LTILE = 512
NLT = L // LTILE            # 8 l-tiles
LEXT = L + 2                # extended h1 columns
EWC = K * HID               # fused table row length (384)

GCHUNK = int(os.environ.get("KERNEL_GCHUNK", "8"))
SINGLE_PACKET = os.environ.get("KERNEL_SINGLE_PACKET", "1") == "1"
NQUEUES = int(os.environ.get("KERNEL_NQUEUES", "4"))
XTBUFS = int(os.environ.get("KERNEL_XTBUFS", str(3 * GCHUNK)))
CSPAN = L // GCHUNK
CNIDX = CSPAN + 128
NVALID = CSPAN + 2   # real indices per chunk; the rest are -1 (ucode trims)
# HW limit: the single-packet tx stream allows <=64 descriptors/engine,
# i.e. num_idxs/16 + 2 <= 64 -> num_idxs <= 992 per dma_gather.
assert CNIDX <= 992


def _dtype_np(mode):
    import ml_dtypes

    return np.float16 if mode == "f16" else ml_dtypes.bfloat16


def _dtype_my(mode):
    import concourse.mybir as mybir

    return mybir.dt.float16 if mode == "f16" else mybir.dt.bfloat16


def build_program(mode="f16", nbatch=BLOC):
    """Build the per-core Bass program."""
    import concourse.bacc as bacc
    import concourse.mybir as mybir
    import concourse.tile as tile
    from concourse._compat import get_trn_type

    f32 = mybir.dt.float32
    f16 = _dtype_my(mode)
    i16 = mybir.dt.int16
    RELU = mybir.ActivationFunctionType.Relu
    IDENT = mybir.ActivationFunctionType.Identity
    AX = mybir.AxisListType.X

    nc = bacc.Bacc(
        get_trn_type() or "TRN2",
        target_bir_lowering=False,
        debug=False,
        enable_asserts=False,
        num_devices=NCORES,
        num_swdge_queues=NQUEUES,
        # ring capacity per queue = scratch/16 descs; all 8 gathers of a
        # queue (134 s2m descs each) must fit to avoid reclaim stalls.
        dynamic_dma_scratch_size=49152,
    )

    ncols = CNIDX // 16  # idx columns per gather chunk

    ew_d = nc.dram_tensor("ew", [VOCAB, EWC], f16, kind="ExternalInput")
    idx_d = nc.dram_tensor("idx", [128, nbatch * GCHUNK * ncols], i16,
                           kind="ExternalInput")
    id_d = nc.dram_tensor("id128", [128, 128], f16, kind="ExternalInput")
    w2_d = nc.dram_tensor("w2t", [128, K * HID], f16, kind="ExternalInput")
    b1_d = nc.dram_tensor("b1c", [128, 1], f32, kind="ExternalInput")
    b2_d = nc.dram_tensor("b2c", [128, 1], f32, kind="ExternalInput")
    lw1_d = nc.dram_tensor("lw1t", [128, HID], f16, kind="ExternalInput")
    lb1_d = nc.dram_tensor("lb1c", [128, 1], f32, kind="ExternalInput")
    lw2_d = nc.dram_tensor("lw2t", [128, CLASSES], f16, kind="ExternalInput")
    lb2_d = nc.dram_tensor("lb2c", [CLASSES, 1], f32, kind="ExternalInput")
    out_d = nc.dram_tensor("out", [CLASSES, nbatch], f32, kind="ExternalOutput")

    from concourse import library_config

    with tile.TileContext(nc) as tc:
        # preload the GPSIMD ucode library that dma_gather needs, so its
        # ~9us IRAM fetch overlaps the preamble instead of stalling the
        # first gather.
        nc.gpsimd.load_library(library_config.mlp)
        with (
            tc.tile_pool(name="const", bufs=1) as cpool,
            tc.tile_pool(name="xt", bufs=XTBUFS) as xt_pool,
            tc.tile_pool(name="h1", bufs=2) as h1_pool,
            tc.tile_pool(name="small", bufs=2) as sm_pool,
            tc.tile_pool(name="ps", bufs=6, space="PSUM") as ps_pool,
            tc.tile_pool(name="psm", bufs=1, space="PSUM") as psm_pool,
        ):
            id_sb = cpool.tile([128, 128], f16)
            nc.sync.dma_start(id_sb[:, :], id_d.ap())
            w2_sb = cpool.tile([128, K * HID], f16)
            nc.sync.dma_start(w2_sb[:, :], w2_d.ap())
            lw1_sb = cpool.tile([128, HID], f16)
            nc.sync.dma_start(lw1_sb[:, :], lw1_d.ap())
            lw2_sb = cpool.tile([128, CLASSES], f16)
            nc.sync.dma_start(lw2_sb[:, :], lw2_d.ap())
            b1_sb = cpool.tile([128, 1], f32)
            nc.sync.dma_start(b1_sb[:, :], b1_d.ap())
            b2_sb = cpool.tile([128, 1], f32)
            nc.sync.dma_start(b2_sb[:, :], b2_d.ap())
            lb1_sb = cpool.tile([128, 1], f32)
            nc.sync.dma_start(lb1_sb[:, :], lb1_d.ap())
            lb2_sb = cpool.tile([CLASSES, 1], f32)
            nc.sync.dma_start(lb2_sb[:, :], lb2_d.ap())
            idx_sb = cpool.tile([128, nbatch * GCHUNK * ncols], i16)
            nc.sync.dma_start(idx_sb[:, :], idx_d.ap())

            y_sb = cpool.tile([128, nbatch], f16, tag="ytile")

            for b in range(nbatch):
                xts = []
                for c in range(GCHUNK):
                    xt = xt_pool.tile([128, K, CNIDX], f16, tag="xt")
                    o = (b * GCHUNK + c) * ncols
                    nc.gpsimd.dma_gather(
                        out_ap=xt[:, :, :],
                        in_ap=ew_d.ap(),
                        idxs_ap=idx_sb[:, o : o + ncols],
                        num_idxs=CNIDX,
                        num_idxs_reg=NVALID,
                        elem_size=EWC,
                        transpose=True,
                        single_packet=SINGLE_PACKET,
                        queue_num=(b * GCHUNK + c) % NQUEUES,
                    )
                    xts.append(xt)

                h1 = h1_pool.tile([128, LEXT], f16, tag="h1")
                for lt in range(NLT):
                    l0 = lt * LTILE
                    xt = xts[l0 // CSPAN]
                    base = l0 % CSPAN
                    ps1 = ps_pool.tile([128, LTILE], f32, tag="ps")
                    # conv1 output = sum of k-shifted gathered slices
                    for k in range(K):
                        nc.tensor.matmul(
                            ps1[:, :],
                            lhsT=id_sb[:, :],
                            rhs=xt[:, k, base + 2 - k : base + 2 - k + LTILE],
                            start=(k == 0),
                            stop=(k == K - 1),
                        )
                    nc.scalar.activation(
                        h1[:, 2 + l0 : 2 + l0 + LTILE], ps1[:, :], RELU,
                        bias=b1_sb[:, 0:1],
                    )

                # circular wrap: h1[-1], h1[-2] live at ext cols 1, 0
                nc.scalar.copy(h1[:, 0:2], h1[:, L : L + 2])

                mx = sm_pool.tile([128, NLT], f32, tag="mx")
                # tile 0 last: it needs the wrap columns, which depend on
                # conv1's final tile - every other tile is ready earlier
                for lt in list(range(1, NLT)) + [0]:
                    l0 = lt * LTILE
                    ps2 = ps_pool.tile([128, LTILE], f32, tag="ps")
                    for k in range(K):
                        nc.tensor.matmul(
                            ps2[:, :],
                            lhsT=w2_sb[:, k * HID : (k + 1) * HID],
                            rhs=h1[:, l0 + 2 - k : l0 + 2 - k + LTILE],
                            start=(k == 0),
                            stop=(k == K - 1),
                        )
                    nc.vector.reduce_max(mx[:, lt : lt + 1], ps2[:, :], axis=AX)

                pooled = sm_pool.tile([128, 1], f32, tag="pooled")
                nc.vector.reduce_max(pooled[:, :], mx[:, :], axis=AX)
                # max-pool commutes with (+b2, relu); cast to f16 for the MLP
                nc.scalar.activation(
                    y_sb[:, b : b + 1], pooled[:, :], RELU, bias=b2_sb[:, 0:1]
                )

            # --- tiny MLP head on all nbatch columns at once ---
            psm1 = psm_pool.tile([128, nbatch], f32, tag="psm1")
            nc.tensor.matmul(psm1[:, :], lhsT=lw1_sb[:, :], rhs=y_sb[:, :],
                             start=True, stop=True)
            z1 = sm_pool.tile([128, nbatch], f16, tag="z1")
            nc.scalar.activation(z1[:, :], psm1[:, :], RELU, bias=lb1_sb[:, 0:1])

            psm2 = psm_pool.tile([CLASSES, nbatch], f32, tag="psm2")
            nc.tensor.matmul(psm2[:, :], lhsT=lw2_sb[:, :], rhs=z1[:, :],
                             start=True, stop=True)
            out_sb = sm_pool.tile([CLASSES, nbatch], f32, tag="osb")
            nc.scalar.activation(out_sb[:, :], psm2[:, :], IDENT,
                                 bias=lb2_sb[:, 0:1])
            nc.sync.dma_start(out_d.ap(), out_sb[:, :])

    nc.compile()
    return nc


def prep_host_inputs(tokens, emb, w1, b1, w2, b2, lw1, lb1, lw2, lb2,
                     mode="f16", nbatch=BLOC):
    """Host-side layout prep.  Returns per-core in_maps."""
    npdt = _dtype_np(mode)
    tokens = np.asarray(tokens).astype(np.int64)
    emb = np.asarray(emb, np.float32)
    w1 = np.asarray(w1, np.float32)               # [HID, EMB, K]

    # fused conv1 table: ew[t, k*HID + o] = sum_c emb[t, c] * w1[o, c, k]
    ew = np.empty((VOCAB, EWC), np.float32)
    for k in range(K):
        ew[:, k * HID : (k + 1) * HID] = emb @ w1[:, :, k].T
    ew = np.ascontiguousarray(ew.astype(npdt))

    # w2t[p, k*HID + o] = w2[o, p, k]
    w2 = np.asarray(w2, np.float32)               # [HID, HID, K]
    w2t = np.ascontiguousarray(
        w2.transpose(1, 2, 0).reshape(128, K * HID).astype(npdt)
    )
    lw1t = np.ascontiguousarray(np.asarray(lw1, np.float32).T.astype(npdt))
    lw2t = np.ascontiguousarray(np.asarray(lw2, np.float32).T.astype(npdt))
    b1c = np.asarray(b1, np.float32).reshape(128, 1)
    b2c = np.asarray(b2, np.float32).reshape(128, 1)
    lb1c = np.asarray(lb1, np.float32).reshape(128, 1)
    lb2c = np.asarray(lb2, np.float32).reshape(CLASSES, 1)
    id128 = np.eye(128, dtype=np.float32).astype(npdt)

    pos = np.arange(CNIDX)
    in_maps = []
    for c in range(NCORES):
        idx_cols = []
        for j in range(nbatch):
            t = tokens[c * BLOC + j]
            for g in range(GCHUNK):
                ext = t[(g * CSPAN - 2 + pos) % L].astype(np.int16)
                ext[NVALID:] = -1  # ucode trims trailing -1s: no wasted bytes
                # gathered row i reads idxs[16, ncols] at [i % 16, i // 16]
                wrapped = ext.reshape(CNIDX // 16, 16).T
                idx_cols.append(np.tile(wrapped, (8, 1)))      # [128, ncols]
        idx = np.ascontiguousarray(np.concatenate(idx_cols, axis=1))
        in_maps.append({
            "ew": ew, "idx": idx, "id128": id128, "w2t": w2t,
            "b1c": b1c, "b2c": b2c, "lw1t": lw1t, "lb1c": lb1c,
            "lw2t": lw2t, "lb2c": lb2c,
        })
    return in_maps


_CACHE = {}


def _get_program(mode):
    if mode not in _CACHE:
        _CACHE[mode] = build_program(mode)
    return _CACHE[mode]


def run(inputs, mode=None, trace=False, trace_kwargs=None):
    """Run on 8 cores; returns (output[32, 6] f32, BassKernelResults)."""
    from concourse import bass_utils

    mode = mode or os.environ.get("KERNEL_MODE", "f16")
    nc = _get_program(mode)
    in_maps = prep_host_inputs(**inputs, mode=mode)
    res = bass_utils.run_bass_kernel_spmd(
        nc, in_maps, core_ids=list(range(NCORES)), trace=trace,
        **(trace_kwargs or {}),
    )
    out = np.empty((B, CLASSES), np.float32)
    for c in range(NCORES):
        o = res.results[c]["out"]  # [CLASSES, BLOC]
        out[c * BLOC : (c + 1) * BLOC, :] = np.asarray(o, np.float32).T
    return out, res


def kernel(**inputs):
    out, _ = run(inputs)
    return out
